# revision 2
# baseline (speedup 1.0000x reference)
"""MDCA loss kernel for Trainium2, 8 NeuronCores, data-parallel over batch.

reference:
    counts[c]   = histogram(target) ; avg_count = counts/B
    avg_conf    = mean(logits, axis=1)            # [E, C]
    loss[e]     = mean_c |avg_conf[e,c] - avg_count[c]|

Strategy per core (batch shard of 1024 rows, partition p holds rows 8p..8p+7):
  - the 16.4 MB logits shard streams over the 3 DMA queues (sync/scalar
    HWDGE + gpsimd SWDGE) as descending-size chunks: 2 MB, 2 MB, 1 MB per
    queue, then two 0.25 MB half-row chunks on each HWDGE queue.  Every
    chunk has its own SBUF buffer so no ring ever stalls on reuse; all
    dma_starts are emitted first so the rings stay full end-to-end.
    (HBM QoS throttles the stream to ~330 GB/s after ~15 us — wire floor.)
  - histogram: iota + tensor_scalar(is_equal) one-hots (bf16), matmul with
    [128,4] -1 weights opens the PSUM accumulation chain per column half
    with -count broadcast to all 4 exit rows
  - conf: DVE folds each 4-row chunk 4->2 row-groups (tensor_add, f32r out)
    and 2-row chunks 2->1; f32r PE matmuls with a [128,4] selector (ones in
    column e) fold rows+partitions into the SAME PSUM chain, so
    psum[e,c] = sum_conf[e,c] - count[c] with no separate combine step
  - the final half-row chunks are DMA'd straight into f32r tiles (bitcast)
    and fed to one closing matmul per chain: tail after the last HBM byte
    is ~1 us of PE + 2 tiny PSUM->SBUF copies + two parallel 8 KB stores
  - host sums the 8 per-core partials and takes |.|-mean / (B*C) -> loss[4]
    (an on-device AllReduce costs ~35 us for 16 KB; host finish wins)
"""

import os
import sys

for _p in ("/opt/trn_rl_repo", "/root/.axon_site/_ro/trn_rl_repo"):
    if os.path.isdir(_p) and _p not in sys.path:
        sys.path.insert(0, _p)

import numpy as np

import concourse.bass as bass
import concourse.bacc as bacc
import concourse.tile as tile
import concourse.mybir as mybir
from concourse.bass_utils import run_bass_kernel_spmd

E, B, C = 4, 8192, 1000
N_CORES = 8
BS = B // N_CORES          # 1024 batch rows per core
GP = 8                     # rows folded per partition (BS = 128 * GP)
CH = C // 2                # 500, C half per PSUM bank
F32 = mybir.dt.float32
F32R = mybir.dt.float32r
BF16 = mybir.dt.bfloat16


def build_nc():
    nc = bacc.Bacc(
        "TRN2",
        target_bir_lowering=False,
        debug=False,
        enable_asserts=False,
        num_devices=N_CORES,
    )

    logits = nc.dram_tensor("logits", [E, BS, C], F32, kind="ExternalInput")
    # host pre-arranges the target shard as [128, GP] float32 (exact ints)
    target = nc.dram_tensor("target_f", [128, GP], F32, kind="ExternalInput")
    part_out = nc.dram_tensor("part", [E, C], F32, kind="ExternalOutput")

    # per-exit view: partition p holds rows 8p..8p+7
    src = [logits[e].rearrange("(p i) c -> p i c", i=GP) for e in range(E)]

    with tile.TileContext(nc) as tc:
        with (
            tc.tile_pool(name="const", bufs=1) as const,
            tc.tile_pool(name="ldA", bufs=6) as ldA,
            tc.tile_pool(name="ldC", bufs=3) as ldC,
            tc.tile_pool(name="ldQ", bufs=4) as ldQ,
            tc.tile_pool(name="fA", bufs=3) as fAp,
            tc.tile_pool(name="fC", bufs=2) as fCp,
            tc.tile_pool(name="work", bufs=3) as work,
            tc.tile_pool(name="psum", bufs=1, space=bass.MemorySpace.PSUM) as psum,
        ):
            # ---- phase 1: every load DMA, emitted first so the three
            # rings fill immediately and drain back-to-back.
            # A chunks: 2 MB (rows 0-3 or 4-7 of one exit), 16 KB lines
            # C chunks: 1 MB (2 rows of exit 3)
            # Q chunks: 0.25 MB (half of one row of exit 3) -> f32r direct
            def ld_dma(eng, pool, tag, e, r0, r1, c0=0, c1=C, dt=F32):
                rows = r1 - r0
                t = pool.tile([128, rows * (c1 - c0)], dt, tag=tag,
                              name=f"{tag}_e{e}r{r0}")
                in_ = src[e][:, r0:r1, c0:c1]
                if dt is F32R:
                    in_ = in_.bitcast(F32R)
                eng.dma_start(
                    out=t.rearrange("p (i c) -> p i c", i=rows), in_=in_
                )
                return t

            tA_s1 = ld_dma(nc.sync, ldA, "ldA", 0, 0, 4)
            tB_s = ld_dma(nc.sync, ldA, "ldA", 1, 4, 8)
            tC_s = ld_dma(nc.sync, ldC, "ldC", 3, 0, 2)
            tE_s = ld_dma(nc.sync, ldQ, "ldQ", 3, 6, 7, 0, CH, F32R)
            tF_s = ld_dma(nc.sync, ldQ, "ldQ", 3, 7, 8, 0, CH, F32R)

            tgt_sb = const.tile([128, GP], F32, tag="tgt")
            nc.scalar.dma_start(out=tgt_sb[:], in_=target[:])
            tA_c = ld_dma(nc.scalar, ldA, "ldA", 1, 0, 4)
            tB_c = ld_dma(nc.scalar, ldA, "ldA", 2, 4, 8)
            tC_c = ld_dma(nc.scalar, ldC, "ldC", 3, 2, 4)
            tE_c = ld_dma(nc.scalar, ldQ, "ldQ", 3, 6, 7, CH, C, F32R)
            tF_c = ld_dma(nc.scalar, ldQ, "ldQ", 3, 7, 8, CH, C, F32R)

            tA_g = ld_dma(nc.gpsimd, ldA, "ldA", 0, 4, 8)
            tB_g = ld_dma(nc.gpsimd, ldA, "ldA", 2, 0, 4)
            tC_g = ld_dma(nc.gpsimd, ldC, "ldC", 3, 4, 6)

            # ---- phase 2: constants (DVE + gpsimd, off the DMA rings)
            # selector weights: sels[:, 4e:4e+4] has ones in column e
            # (f32r so the PE folds run at 1 cyc/row; memset can't write
            # f32r, so build in f32 and convert)
            sels_f = const.tile([128, 4 * E], F32, tag="sels_f")
            nc.vector.memset(sels_f[:], 0.0)
            for e in range(E):
                nc.vector.memset(sels_f[:, 4 * e + e : 4 * e + e + 1], 1.0)
            sels = const.tile([128, 4 * E], F32R, tag="sels")
            nc.vector.tensor_copy(sels[:], sels_f[:])
            neg1 = const.tile([128, E], BF16, tag="neg1")
            nc.vector.memset(neg1[:], -1.0)
            iota_f = const.tile([128, C], F32, tag="iota")
            nc.gpsimd.iota(
                iota_f[:],
                pattern=[[1, C]],
                base=0,
                channel_multiplier=0,
                allow_small_or_imprecise_dtypes=True,
            )

            # one merged PSUM accumulation chain per column half:
            # opened by the histogram matmuls, closed by the final
            # half-row conf matmuls -> psum[e,c] = conf_sum - count
            pbank = [
                psum.tile([E, CH], F32, tag=f"pc{h}", name=f"pc{h}")
                for h in range(2)
            ]

            # ---- phase 3: histogram one-hots + chain-opening matmuls
            # (ready long before the first 2 MB chunk lands)
            for i in range(GP):
                onehot = work.tile([128, C], BF16, tag="onehot")
                nc.vector.tensor_scalar(
                    onehot[:],
                    iota_f[:],
                    tgt_sb[:, i : i + 1],
                    None,
                    mybir.AluOpType.is_equal,
                )
                for h in range(2):
                    nc.tensor.matmul(
                        pbank[h][:],
                        neg1[:],
                        onehot[:, h * CH : (h + 1) * CH],
                        start=(i == 0),
                        stop=False,
                    )

            # ---- phase 4: folds + conf matmuls, in expected arrival order
            def fold4(t, e):
                # 4 rows -> 2 row-groups, f32 -> f32r
                f = fAp.tile([128, 2 * C], F32R, tag="fA", name=f"fA_e{e}")
                nc.vector.tensor_add(f[:], t[:, : 2 * C], t[:, 2 * C :])
                for g in range(2):
                    for h in range(2):
                        nc.tensor.matmul(
                            pbank[h][:],
                            sels[:, 4 * e : 4 * e + 4],
                            f[:, g * C + h * CH : g * C + (h + 1) * CH],
                            start=False,
                            stop=False,
                        )

            def fold2(t, e):
                # 2 rows -> 1 row-group
                f = fCp.tile([128, C], F32R, tag="fC", name=f"fC_e{e}")
                nc.vector.tensor_add(f[:], t[:, :C], t[:, C:])
                for h in range(2):
                    nc.tensor.matmul(
                        pbank[h][:],
                        sels[:, 4 * e : 4 * e + 4],
                        f[:, h * CH : (h + 1) * CH],
                        start=False,
                        stop=False,
                    )

            fold4(tA_s1, 0)
            fold4(tA_c, 1)
            fold4(tA_g, 0)
            fold4(tB_s, 1)
            fold4(tB_c, 2)
            fold4(tB_g, 2)
            fold2(tC_s, 3)
            fold2(tC_c, 3)
            fold2(tC_g, 3)

            # half-row closers: f32r tiles fed straight to the PE
            nc.tensor.matmul(pbank[0][:], sels[:, 12:16], tE_s[:],
                             start=False, stop=False)
            nc.tensor.matmul(pbank[1][:], sels[:, 12:16], tE_c[:],
                             start=False, stop=False)
            nc.tensor.matmul(pbank[0][:], sels[:, 12:16], tF_s[:],
                             start=False, stop=True)
            nc.tensor.matmul(pbank[1][:], sels[:, 12:16], tF_c[:],
                             start=False, stop=True)

            # ---- phase 5: two tiny PSUM->SBUF copies + parallel stores
            for h, eng in ((0, nc.sync), (1, nc.scalar)):
                sb = work.tile([E, CH], F32, tag=f"sb{h}")
                nc.vector.tensor_copy(sb[:], pbank[h][:])
                eng.dma_start(out=part_out[:, h * CH : (h + 1) * CH], in_=sb[:])

    nc.compile()
    return nc


_NC_CACHE = {}


def _get_nc():
    if "nc" not in _NC_CACHE:
        _NC_CACHE["nc"] = build_nc()
    return _NC_CACHE["nc"]


def make_in_maps(logits: np.ndarray, target: np.ndarray):
    logits = np.ascontiguousarray(logits, dtype=np.float32)
    target = np.asarray(target)
    in_maps = []
    for c in range(N_CORES):
        lg = logits[:, c * BS : (c + 1) * BS, :]
        tg = target[c * BS : (c + 1) * BS].astype(np.float32).reshape(128, GP)
        in_maps.append({"logits": np.ascontiguousarray(lg), "target_f": tg})
    return in_maps


def kernel(logits: np.ndarray, target: np.ndarray) -> np.ndarray:
    nc = _get_nc()
    in_maps = make_in_maps(logits, target)
    res = run_bass_kernel_spmd(nc, in_maps, core_ids=list(range(N_CORES)))
    parts = sum(np.asarray(r["part"], dtype=np.float64) for r in res.results)
    return (np.abs(parts).sum(axis=1) / (B * C)).astype(np.float32)


# revision 3
# speedup vs baseline: 1.0746x; 1.0746x over previous
"""MDCA loss kernel for Trainium2, 8 NeuronCores, data-parallel over batch.

reference:
    counts[c]   = histogram(target) ; avg_count = counts/B
    avg_conf    = mean(logits, axis=1)            # [E, C]
    loss[e]     = mean_c |avg_conf[e,c] - avg_count[c]|

Strategy per core (batch shard of 1024 rows, partition p holds rows 8p..8p+7):
  - the 16.4 MB logits shard streams over the 3 DMA queues (sync/scalar
    HWDGE + gpsimd SWDGE).  Lines of 16 KB (4 contiguous rows per
    partition) hit the SDMA engines' best per-descriptor rate (~26 GB/s
    each); smaller lines cap at ~19 GB/s, so everything is 2 MB chunks
    except the two closing 1 MB chunks on sync (8 KB lines) which land
    last and keep the post-stream compute tiny.  All dma_starts are
    emitted first so the rings stay full end-to-end.  (HBM QoS throttles
    the stream to ~330 GB/s after ~15 us — that is the wire floor.)
  - target arrives as [128, 128] f32 (each of the 8 row-targets
    replicated 16x -> 512 B lines, avoiding 32 B read-modify-write
    descriptors); histogram one-hots via iota + tensor_scalar(is_equal)
  - ONE PSUM accumulation chain per 500-column half: opened by the
    histogram matmuls ([128,4] -1 weights broadcast -count to the 4 exit
    rows), continued by f32r conf matmuls ([128,4] selector with ones in
    column e folds rows+partitions), closed by the last chunk's matmuls:
    psum[e,c] = sum_conf[e,c] - count[c] with no combine step
  - DVE folds each chunk's rows pairwise into [128, 1000] f32r tiles
    (narrow-pitch rhs: 390 ns matmuls vs 620 ns from [128,2000] tiles)
  - tail after the last HBM byte: one fold + 2 matmuls + 2 tiny
    PSUM->SBUF copies + two parallel 8 KB stores
  - host sums the 8 per-core partials and takes |.|-mean / (B*C) -> loss[4]
    (an on-device AllReduce costs ~35 us for 16 KB; host finish wins)
"""

import os
import sys

for _p in ("/opt/trn_rl_repo", "/root/.axon_site/_ro/trn_rl_repo"):
    if os.path.isdir(_p) and _p not in sys.path:
        sys.path.insert(0, _p)

import numpy as np

import concourse.bass as bass
import concourse.bacc as bacc
import concourse.tile as tile
import concourse.mybir as mybir
from concourse.bass_utils import run_bass_kernel_spmd

E, B, C = 4, 8192, 1000
N_CORES = 8
BS = B // N_CORES          # 1024 batch rows per core
GP = 8                     # rows folded per partition (BS = 128 * GP)
CH = C // 2                # 500, C half per PSUM bank
REP = 16                   # target replication -> 512 B DMA lines
F32 = mybir.dt.float32
F32R = mybir.dt.float32r
BF16 = mybir.dt.bfloat16


def build_nc():
    nc = bacc.Bacc(
        "TRN2",
        target_bir_lowering=False,
        debug=False,
        enable_asserts=False,
        num_devices=N_CORES,
    )

    logits = nc.dram_tensor("logits", [E, BS, C], F32, kind="ExternalInput")
    target = nc.dram_tensor("target_f", [128, GP * REP], F32,
                            kind="ExternalInput")
    part_out = nc.dram_tensor("part", [E, C], F32, kind="ExternalOutput")

    # per-exit view: partition p holds rows 8p..8p+7
    src = [logits[e].rearrange("(p i) c -> p i c", i=GP) for e in range(E)]

    with tile.TileContext(nc) as tc:
        with (
            tc.tile_pool(name="const", bufs=1) as const,
            tc.tile_pool(name="ld2", bufs=7) as ld2,
            tc.tile_pool(name="ld1", bufs=2) as ld1,
            tc.tile_pool(name="fold", bufs=4) as foldp,
            tc.tile_pool(name="work", bufs=3) as work,
            tc.tile_pool(name="psum", bufs=1, space=bass.MemorySpace.PSUM) as psum,
        ):
            # ---- phase 1: every load DMA first, so the three rings fill
            # immediately and drain back-to-back
            def ld_dma(eng, pool, tag, e, r0, r1):
                rows = r1 - r0
                t = pool.tile([128, rows * C], F32, tag=tag,
                              name=f"{tag}_e{e}r{r0}")
                eng.dma_start(
                    out=t.rearrange("p (i c) -> p i c", i=rows),
                    in_=src[e][:, r0:r1, :],
                )
                return t

            tA_s = ld_dma(nc.sync, ld2, "ld2", 0, 0, 4)
            tB_s = ld_dma(nc.sync, ld2, "ld2", 1, 4, 8)
            tC1_s = ld_dma(nc.sync, ld1, "ld1", 3, 0, 2)
            tC2_s = ld_dma(nc.sync, ld1, "ld1", 3, 2, 4)

            tgt_sb = const.tile([128, GP * REP], F32, tag="tgt")
            nc.scalar.dma_start(out=tgt_sb[:], in_=target[:])
            tA_c = ld_dma(nc.scalar, ld2, "ld2", 1, 0, 4)
            tB_c = ld_dma(nc.scalar, ld2, "ld2", 2, 4, 8)
            tC_c = ld_dma(nc.scalar, ld2, "ld2", 3, 4, 8)

            tA_g = ld_dma(nc.gpsimd, ld2, "ld2", 0, 4, 8)
            tB_g = ld_dma(nc.gpsimd, ld2, "ld2", 2, 0, 4)

            # ---- phase 2: constants (DVE + gpsimd, off the DMA rings)
            sels_f = const.tile([128, 4 * E], F32, tag="sels_f")
            nc.vector.memset(sels_f[:], 0.0)
            for e in range(E):
                nc.vector.memset(sels_f[:, 4 * e + e : 4 * e + e + 1], 1.0)
            sels = const.tile([128, 4 * E], F32R, tag="sels")
            nc.vector.tensor_copy(sels[:], sels_f[:])
            neg1 = const.tile([128, E], BF16, tag="neg1")
            nc.vector.memset(neg1[:], -1.0)
            iota_f = const.tile([128, C], F32, tag="iota")
            nc.gpsimd.iota(
                iota_f[:],
                pattern=[[1, C]],
                base=0,
                channel_multiplier=0,
                allow_small_or_imprecise_dtypes=True,
            )

            # one merged PSUM accumulation chain per column half
            pbank = [
                psum.tile([E, CH], F32, tag=f"pc{h}", name=f"pc{h}")
                for h in range(2)
            ]

            # ---- phase 3: histogram one-hots + chain-opening matmuls
            for i in range(GP):
                onehot = work.tile([128, C], BF16, tag="onehot")
                nc.vector.tensor_scalar(
                    onehot[:],
                    iota_f[:],
                    tgt_sb[:, REP * i : REP * i + 1],
                    None,
                    mybir.AluOpType.is_equal,
                )
                for h in range(2):
                    nc.tensor.matmul(
                        pbank[h][:],
                        neg1[:],
                        onehot[:, h * CH : (h + 1) * CH],
                        start=(i == 0),
                        stop=False,
                    )

            # ---- phase 4: folds + conf matmuls in expected arrival order.
            # each fold makes a narrow [128, C] f32r tile (fast mm rhs)
            def mm(f, e, stop=False):
                for h in range(2):
                    nc.tensor.matmul(
                        pbank[h][:],
                        sels[:, 4 * e : 4 * e + 4],
                        f[:, h * CH : (h + 1) * CH],
                        start=False,
                        stop=stop,
                    )

            def fold4(t, e):
                # 4 rows -> two [128, C] row-group tiles
                for g in range(2):
                    f = foldp.tile([128, C], F32R, tag="fold",
                                   name=f"f_e{e}g{g}")
                    nc.vector.tensor_add(
                        f[:], t[:, 2 * g * C : (2 * g + 1) * C],
                        t[:, (2 * g + 1) * C : (2 * g + 2) * C],
                    )
                    mm(f, e)

            def fold2(t, e, stop=False):
                f = foldp.tile([128, C], F32R, tag="fold", name=f"f2_e{e}")
                nc.vector.tensor_add(f[:], t[:, :C], t[:, C:])
                mm(f, e, stop=stop)

            fold4(tA_s, 0)
            fold4(tA_c, 1)
            fold4(tA_g, 0)
            fold4(tB_s, 1)
            fold4(tB_c, 2)
            fold4(tB_g, 2)
            fold4(tC_c, 3)
            fold2(tC1_s, 3)
            fold2(tC2_s, 3, stop=True)

            # ---- phase 5: two tiny PSUM->SBUF copies + parallel stores
            for h, eng in ((0, nc.sync), (1, nc.scalar)):
                sb = work.tile([E, CH], F32, tag=f"sb{h}")
                nc.vector.tensor_copy(sb[:], pbank[h][:])
                eng.dma_start(out=part_out[:, h * CH : (h + 1) * CH], in_=sb[:])

    nc.compile()
    return nc


_NC_CACHE = {}


def _get_nc():
    if "nc" not in _NC_CACHE:
        _NC_CACHE["nc"] = build_nc()
    return _NC_CACHE["nc"]


def make_in_maps(logits: np.ndarray, target: np.ndarray):
    logits = np.ascontiguousarray(logits, dtype=np.float32)
    target = np.asarray(target)
    in_maps = []
    for c in range(N_CORES):
        lg = logits[:, c * BS : (c + 1) * BS, :]
        tg = np.repeat(
            target[c * BS : (c + 1) * BS].astype(np.float32).reshape(128, GP),
            REP, axis=1,
        )
        in_maps.append(
            {"logits": np.ascontiguousarray(lg),
             "target_f": np.ascontiguousarray(tg)}
        )
    return in_maps


def kernel(logits: np.ndarray, target: np.ndarray) -> np.ndarray:
    nc = _get_nc()
    in_maps = make_in_maps(logits, target)
    res = run_bass_kernel_spmd(nc, in_maps, core_ids=list(range(N_CORES)))
    parts = sum(np.asarray(r["part"], dtype=np.float64) for r in res.results)
    return (np.abs(parts).sum(axis=1) / (B * C)).astype(np.float32)


# revision 5
# speedup vs baseline: 1.0952x; 1.0192x over previous
"""MDCA loss kernel for Trainium2, 8 NeuronCores, data-parallel over batch.

reference:
    counts[c]   = histogram(target) ; avg_count = counts/B
    avg_conf    = mean(logits, axis=1)            # [E, C]
    loss[e]     = mean_c |avg_conf[e,c] - avg_count[c]|

Strategy per core (batch shard of 1024 rows, partition p holds rows 8p..8p+7):
  - the 16.4 MB logits shard streams over the 3 DMA queues (sync/scalar
    HWDGE + gpsimd SWDGE).  16 KB lines (4 contiguous rows/partition) hit
    the SDMA engines' best per-descriptor rate (~26 GB/s each; the HBM
    QoS governor caps the aggregate at ~330 GB/s after ~15 us).  All
    dma_starts are emitted first so the rings stay full end-to-end.
  - ring FIFO makes sync the closer: it carries 1 MB more than the other
    queues and ends with four 0.5 MB single-row chunks DMA'd straight
    into f32r tiles (bitcast).  Those need NO DVE fold -- just two 390 ns
    PE matmuls each -- so the post-stream tail is ~1 us of PE, while the
    other queues' last chunks fold during the closers' drain.
  - target arrives as [128, 128] f32 (each row-target replicated 16x ->
    512 B lines, no read-modify-write descriptors), first on gpsimd
  - ONE PSUM accumulation chain per 500-column half: opened by the
    histogram matmuls ([128,4] -1 weights broadcast -count to the 4 exit
    rows), continued by f32r conf matmuls ([128,4] selector with ones in
    column e folds rows+partitions), closed by the last closer chunk:
    psum[e,c] = sum_conf[e,c] - count[c] with no combine step
  - DVE folds 4-row chunks pairwise into [128, 1000] f32r tiles
    (narrow-pitch rhs: 390 ns matmuls vs 620 ns from [128,2000] tiles)
  - tail: 2 matmuls, then PSUM->SBUF copies on DVE (h0) and ACT (h1) in
    parallel, then two parallel 8 KB stores on the two HWDGE rings
  - host sums the 8 per-core partials and takes |.|-mean / (B*C) -> loss[4]
    (an on-device AllReduce costs ~35 us for 16 KB; host finish wins)
"""

import os
import sys

for _p in ("/opt/trn_rl_repo", "/root/.axon_site/_ro/trn_rl_repo"):
    if os.path.isdir(_p) and _p not in sys.path:
        sys.path.insert(0, _p)

import numpy as np

import concourse.bass as bass
import concourse.bacc as bacc
import concourse.tile as tile
import concourse.mybir as mybir
from concourse.bass_utils import run_bass_kernel_spmd

E, B, C = 4, 8192, 1000
N_CORES = 8
BS = B // N_CORES          # 1024 batch rows per core
GP = 8                     # rows folded per partition (BS = 128 * GP)
CH = C // 2                # 500, C half per PSUM bank
REP = 16                   # target replication -> 512 B DMA lines
F32 = mybir.dt.float32
F32R = mybir.dt.float32r
BF16 = mybir.dt.bfloat16


def build_nc():
    nc = bacc.Bacc(
        "TRN2",
        target_bir_lowering=False,
        debug=False,
        enable_asserts=False,
        num_devices=N_CORES,
    )

    logits = nc.dram_tensor("logits", [E, BS, C], F32, kind="ExternalInput")
    target = nc.dram_tensor("target_f", [128, GP * REP], F32,
                            kind="ExternalInput")
    part_out = nc.dram_tensor("part", [E, C], F32, kind="ExternalOutput")

    # per-exit view: partition p holds rows 8p..8p+7
    src = [logits[e].rearrange("(p i) c -> p i c", i=GP) for e in range(E)]

    with tile.TileContext(nc) as tc:
        with (
            tc.tile_pool(name="const", bufs=1) as const,
            tc.tile_pool(name="ld2", bufs=6) as ld2,
            tc.tile_pool(name="ld1", bufs=2) as ld1,
            tc.tile_pool(name="ldz", bufs=4) as ldz,
            tc.tile_pool(name="fold", bufs=4) as foldp,
            tc.tile_pool(name="work", bufs=3) as work,
            tc.tile_pool(name="psum", bufs=1, space=bass.MemorySpace.PSUM) as psum,
        ):
            # ---- phase 1: every load DMA first, so the three rings fill
            # immediately and drain back-to-back
            def ld_dma(eng, pool, tag, e, r0, r1, dt=F32):
                rows = r1 - r0
                t = pool.tile([128, rows * C], dt, tag=tag,
                              name=f"{tag}_e{e}r{r0}")
                in_ = src[e][:, r0:r1, :]
                if dt is F32R:
                    in_ = in_.bitcast(F32R)
                eng.dma_start(
                    out=t.rearrange("p (i c) -> p i c", i=rows), in_=in_
                )
                return t

            tA_s = ld_dma(nc.sync, ld2, "ld2", 0, 0, 4)
            tB_s = ld_dma(nc.sync, ld2, "ld2", 1, 4, 8)
            tZ = [ld_dma(nc.sync, ldz, "ldz", 3, 4 + i, 5 + i, F32R)
                  for i in range(4)]

            tA_c = ld_dma(nc.scalar, ld2, "ld2", 1, 0, 4)
            tB_c = ld_dma(nc.scalar, ld2, "ld2", 2, 4, 8)
            tC_c = ld_dma(nc.scalar, ld1, "ld1", 3, 0, 2)

            tgt_sb = const.tile([128, GP * REP], F32, tag="tgt")
            nc.gpsimd.dma_start(out=tgt_sb[:], in_=target[:])
            tA_g = ld_dma(nc.gpsimd, ld2, "ld2", 0, 4, 8)
            tB_g = ld_dma(nc.gpsimd, ld2, "ld2", 2, 0, 4)
            tC_g = ld_dma(nc.gpsimd, ld1, "ld1", 3, 2, 4)

            # ---- phase 2: constants (DVE + gpsimd, off the DMA rings)
            sels_f = const.tile([128, 4 * E], F32, tag="sels_f")
            nc.vector.memset(sels_f[:], 0.0)
            for e in range(E):
                nc.vector.memset(sels_f[:, 4 * e + e : 4 * e + e + 1], 1.0)
            sels = const.tile([128, 4 * E], F32R, tag="sels")
            nc.vector.tensor_copy(sels[:], sels_f[:])
            neg1 = const.tile([128, E], BF16, tag="neg1")
            nc.vector.memset(neg1[:], -1.0)
            iota_f = const.tile([128, C], F32, tag="iota")
            nc.gpsimd.iota(
                iota_f[:],
                pattern=[[1, C]],
                base=0,
                channel_multiplier=0,
                allow_small_or_imprecise_dtypes=True,
            )

            # one merged PSUM accumulation chain per column half
            pbank = [
                psum.tile([E, CH], F32, tag=f"pc{h}", name=f"pc{h}")
                for h in range(2)
            ]

            # ---- phase 3: histogram one-hots + chain-opening matmuls
            for i in range(GP):
                onehot = work.tile([128, C], BF16, tag="onehot")
                nc.vector.tensor_scalar(
                    onehot[:],
                    iota_f[:],
                    tgt_sb[:, REP * i : REP * i + 1],
                    None,
                    mybir.AluOpType.is_equal,
                )
                for h in range(2):
                    nc.tensor.matmul(
                        pbank[h][:],
                        neg1[:],
                        onehot[:, h * CH : (h + 1) * CH],
                        start=(i == 0),
                        stop=False,
                    )

            # ---- phase 4: folds + conf matmuls in expected arrival order
            def mm(f, e, stop=False):
                for h in range(2):
                    nc.tensor.matmul(
                        pbank[h][:],
                        sels[:, 4 * e : 4 * e + 4],
                        f[:, h * CH : (h + 1) * CH],
                        start=False,
                        stop=stop,
                    )

            def fold4(t, e):
                for g in range(2):
                    f = foldp.tile([128, C], F32R, tag="fold",
                                   name=f"f_e{e}g{g}")
                    nc.vector.tensor_add(
                        f[:], t[:, 2 * g * C : (2 * g + 1) * C],
                        t[:, (2 * g + 1) * C : (2 * g + 2) * C],
                    )
                    mm(f, e)

            def fold2(t, e):
                f = foldp.tile([128, C], F32R, tag="fold", name=f"f2_e{e}")
                nc.vector.tensor_add(f[:], t[:, :C], t[:, C:])
                mm(f, e)

            fold4(tA_s, 0)
            fold4(tA_c, 1)
            fold4(tA_g, 0)
            fold4(tB_s, 1)
            fold4(tB_c, 2)
            fold4(tB_g, 2)
            fold2(tC_c, 3)
            fold2(tC_g, 3)
            # closers: single-row f32r tiles straight to the PE
            mm(tZ[0], 3)
            mm(tZ[1], 3)
            mm(tZ[2], 3)
            mm(tZ[3], 3, stop=True)

            # ---- phase 5: PSUM->SBUF copies on DVE (h0) and ACT (h1),
            # then parallel 8 KB stores on the two HWDGE rings
            sb0 = work.tile([E, CH], F32, tag="sb0")
            nc.vector.tensor_copy(sb0[:], pbank[0][:])
            nc.sync.dma_start(out=part_out[:, 0:CH], in_=sb0[:])
            sb1 = work.tile([E, CH], F32, tag="sb1")
            nc.vector.tensor_copy(sb1[:], pbank[1][:])
            nc.scalar.dma_start(out=part_out[:, CH:C], in_=sb1[:])

    nc.compile()
    return nc


_NC_CACHE = {}


def _get_nc():
    if "nc" not in _NC_CACHE:
        _NC_CACHE["nc"] = build_nc()
    return _NC_CACHE["nc"]


def make_in_maps(logits: np.ndarray, target: np.ndarray):
    logits = np.ascontiguousarray(logits, dtype=np.float32)
    target = np.asarray(target)
    in_maps = []
    for c in range(N_CORES):
        lg = logits[:, c * BS : (c + 1) * BS, :]
        tg = np.repeat(
            target[c * BS : (c + 1) * BS].astype(np.float32).reshape(128, GP),
            REP, axis=1,
        )
        in_maps.append(
            {"logits": np.ascontiguousarray(lg),
             "target_f": np.ascontiguousarray(tg)}
        )
    return in_maps


def kernel(logits: np.ndarray, target: np.ndarray) -> np.ndarray:
    nc = _get_nc()
    in_maps = make_in_maps(logits, target)
    res = run_bass_kernel_spmd(nc, in_maps, core_ids=list(range(N_CORES)))
    parts = sum(np.asarray(r["part"], dtype=np.float64) for r in res.results)
    return (np.abs(parts).sum(axis=1) / (B * C)).astype(np.float32)


# revision 7
# speedup vs baseline: 1.1535x; 1.0533x over previous
"""MDCA loss kernel for Trainium2, 8 NeuronCores, data-parallel over batch.

reference:
    counts[c]   = histogram(target) ; avg_count = counts/B
    avg_conf    = mean(logits, axis=1)            # [E, C]
    loss[e]     = mean_c |avg_conf[e,c] - avg_count[c]|

Strategy per core (batch shard of 1024 rows, partition p holds rows 8p..8p+7):
  - the 16.4 MB logits shard streams over the 3 DMA queues (sync/scalar
    HWDGE + gpsimd SWDGE).  16 KB lines (4 contiguous rows/partition) hit
    the SDMA engines' best per-descriptor rate (~26 GB/s each; the HBM
    QoS governor caps the aggregate at ~330 GB/s after ~15 us).  All
    dma_starts are emitted first so the rings stay full end-to-end.
  - ring FIFO makes sync the closer: it carries 1 MB more than the other
    queues and ends with four 0.5 MB single-row chunks DMA'd straight
    into f32r tiles (bitcast).  Those need NO DVE fold -- just two 390 ns
    PE matmuls each -- so the post-stream tail is ~1 us of PE, while the
    other queues' last chunks fold during the closers' drain.
  - target arrives as [128, 128] f32 (each row-target replicated 16x ->
    512 B lines, no read-modify-write descriptors), first on gpsimd
  - ONE PSUM accumulation chain per 500-column half: opened by the
    histogram matmuls ([128,4] -1 weights broadcast -count to the 4 exit
    rows), continued by f32r conf matmuls ([128,4] selector with ones in
    column e folds rows+partitions), closed by the last closer chunk:
    psum[e,c] = sum_conf[e,c] - count[c] with no combine step
  - DVE folds 4-row chunks pairwise into [128, 1000] f32r tiles
    (narrow-pitch rhs: 390 ns matmuls vs 620 ns from [128,2000] tiles)
  - tail: 2 matmuls, then PSUM->SBUF copies on DVE (h0) and ACT (h1) in
    parallel, then two parallel 8 KB stores on the two HWDGE rings
  - host sums the 8 per-core partials and takes |.|-mean / (B*C) -> loss[4]
    (an on-device AllReduce costs ~35 us for 16 KB; host finish wins)
"""

import os
import sys

for _p in ("/opt/trn_rl_repo", "/root/.axon_site/_ro/trn_rl_repo"):
    if os.path.isdir(_p) and _p not in sys.path:
        sys.path.insert(0, _p)

import numpy as np

import concourse.bass as bass
import concourse.bacc as bacc
import concourse.tile as tile
import concourse.mybir as mybir
from concourse.bass_utils import run_bass_kernel_spmd

E, B, C = 4, 8192, 1000
N_CORES = 8
BS = B // N_CORES          # 1024 batch rows per core
GP = 8                     # rows folded per partition (BS = 128 * GP)
CH = C // 2                # 500, C half per PSUM bank
REP = 16                   # target replication -> 512 B DMA lines
F32 = mybir.dt.float32
F32R = mybir.dt.float32r
BF16 = mybir.dt.bfloat16


def build_nc():
    nc = bacc.Bacc(
        "TRN2",
        target_bir_lowering=False,
        debug=False,
        enable_asserts=False,
        num_devices=N_CORES,
    )

    logits = nc.dram_tensor("logits", [E, BS, C], F32, kind="ExternalInput")
    target = nc.dram_tensor("target_f", [128, GP * REP], F32,
                            kind="ExternalInput")
    part_out = nc.dram_tensor("part", [E, C], F32, kind="ExternalOutput")

    # per-exit view: partition p holds rows 8p..8p+7
    src = [logits[e].rearrange("(p i) c -> p i c", i=GP) for e in range(E)]

    with tile.TileContext(nc) as tc:
        with (
            tc.tile_pool(name="const", bufs=1) as const,
            tc.tile_pool(name="ld2", bufs=6) as ld2,
            tc.tile_pool(name="ld1", bufs=2) as ld1,
            tc.tile_pool(name="ldz", bufs=4) as ldz,
            tc.tile_pool(name="fold", bufs=4) as foldp,
            tc.tile_pool(name="work", bufs=3) as work,
            tc.tile_pool(name="psum", bufs=1, space=bass.MemorySpace.PSUM) as psum,
        ):
            # ---- phase 1: every load DMA first, so the three rings fill
            # immediately and drain back-to-back
            def ld_dma(eng, pool, tag, e, r0, r1, dt=F32):
                rows = r1 - r0
                t = pool.tile([128, rows * C], dt, tag=tag,
                              name=f"{tag}_e{e}r{r0}")
                in_ = src[e][:, r0:r1, :]
                if dt is F32R:
                    in_ = in_.bitcast(F32R)
                eng.dma_start(
                    out=t.rearrange("p (i c) -> p i c", i=rows), in_=in_
                )
                return t

            tA_s = ld_dma(nc.sync, ld2, "ld2", 0, 0, 4)
            tB_s = ld_dma(nc.sync, ld2, "ld2", 1, 4, 8)
            tC_s = ld_dma(nc.sync, ld1, "ld1", 3, 2, 4)
            tZ = [ld_dma(nc.sync, ldz, "ldz", 3, 4 + i, 5 + i, F32R)
                  for i in range(4)]

            tA_c = ld_dma(nc.scalar, ld2, "ld2", 1, 0, 4)
            tB_c = ld_dma(nc.scalar, ld2, "ld2", 2, 4, 8)
            tC_c = ld_dma(nc.scalar, ld1, "ld1", 3, 0, 2)

            tgt_sb = const.tile([128, GP * REP], F32, tag="tgt")
            nc.gpsimd.dma_start(out=tgt_sb[:], in_=target[:])
            tA_g = ld_dma(nc.gpsimd, ld2, "ld2", 0, 4, 8)
            tB_g = ld_dma(nc.gpsimd, ld2, "ld2", 2, 0, 4)

            # ---- phase 2: constants (DVE + gpsimd, off the DMA rings)
            sels_f = const.tile([128, 4 * E], F32, tag="sels_f")
            nc.vector.memset(sels_f[:], 0.0)
            for e in range(E):
                nc.vector.memset(sels_f[:, 4 * e + e : 4 * e + e + 1], 1.0)
            sels = const.tile([128, 4 * E], F32R, tag="sels")
            nc.vector.tensor_copy(sels[:], sels_f[:])
            neg1 = const.tile([128, E], BF16, tag="neg1")
            nc.vector.memset(neg1[:], -1.0)
            iota_f = const.tile([128, C], F32, tag="iota")
            nc.gpsimd.iota(
                iota_f[:],
                pattern=[[1, C]],
                base=0,
                channel_multiplier=0,
                allow_small_or_imprecise_dtypes=True,
            )

            # one merged PSUM accumulation chain per column half
            pbank = [
                psum.tile([E, CH], F32, tag=f"pc{h}", name=f"pc{h}")
                for h in range(2)
            ]

            # ---- phase 3: histogram one-hots + chain-opening matmuls
            for i in range(GP):
                onehot = work.tile([128, C], BF16, tag="onehot")
                nc.vector.tensor_scalar(
                    onehot[:],
                    iota_f[:],
                    tgt_sb[:, REP * i : REP * i + 1],
                    None,
                    mybir.AluOpType.is_equal,
                )
                for h in range(2):
                    nc.tensor.matmul(
                        pbank[h][:],
                        neg1[:],
                        onehot[:, h * CH : (h + 1) * CH],
                        start=(i == 0),
                        stop=False,
                    )

            # ---- phase 4: folds + conf matmuls in expected arrival order
            def mm(f, e, stop=False):
                for h in range(2):
                    nc.tensor.matmul(
                        pbank[h][:],
                        sels[:, 4 * e : 4 * e + 4],
                        f[:, h * CH : (h + 1) * CH],
                        start=False,
                        stop=stop,
                    )

            def fold4(t, e):
                for g in range(2):
                    f = foldp.tile([128, C], F32R, tag="fold",
                                   name=f"f_e{e}g{g}")
                    nc.vector.tensor_add(
                        f[:], t[:, 2 * g * C : (2 * g + 1) * C],
                        t[:, (2 * g + 1) * C : (2 * g + 2) * C],
                    )
                    mm(f, e)

            def fold2(t, e):
                f = foldp.tile([128, C], F32R, tag="fold", name=f"f2_e{e}")
                nc.vector.tensor_add(f[:], t[:, :C], t[:, C:])
                mm(f, e)

            fold4(tA_s, 0)
            fold4(tA_c, 1)
            fold4(tA_g, 0)
            fold4(tB_s, 1)
            fold4(tB_c, 2)
            fold4(tB_g, 2)
            fold2(tC_c, 3)
            fold2(tC_s, 3)
            # closers: single-row f32r tiles straight to the PE
            mm(tZ[0], 3)
            mm(tZ[1], 3)
            mm(tZ[2], 3)
            mm(tZ[3], 3, stop=True)

            # ---- phase 5: PSUM->SBUF copies on DVE (h0) and ACT (h1),
            # then parallel 8 KB stores on the two HWDGE rings
            sb0 = work.tile([E, CH], F32, tag="sb0")
            nc.vector.tensor_copy(sb0[:], pbank[0][:])
            nc.sync.dma_start(out=part_out[:, 0:CH], in_=sb0[:])
            sb1 = work.tile([E, CH], F32, tag="sb1")
            nc.vector.tensor_copy(sb1[:], pbank[1][:])
            nc.scalar.dma_start(out=part_out[:, CH:C], in_=sb1[:])

    nc.compile()
    return nc


_NC_CACHE = {}


def _get_nc():
    if "nc" not in _NC_CACHE:
        _NC_CACHE["nc"] = build_nc()
    return _NC_CACHE["nc"]


def make_in_maps(logits: np.ndarray, target: np.ndarray):
    logits = np.ascontiguousarray(logits, dtype=np.float32)
    target = np.asarray(target)
    in_maps = []
    for c in range(N_CORES):
        lg = logits[:, c * BS : (c + 1) * BS, :]
        tg = np.repeat(
            target[c * BS : (c + 1) * BS].astype(np.float32).reshape(128, GP),
            REP, axis=1,
        )
        in_maps.append(
            {"logits": np.ascontiguousarray(lg),
             "target_f": np.ascontiguousarray(tg)}
        )
    return in_maps


def kernel(logits: np.ndarray, target: np.ndarray) -> np.ndarray:
    nc = _get_nc()
    in_maps = make_in_maps(logits, target)
    res = run_bass_kernel_spmd(nc, in_maps, core_ids=list(range(N_CORES)))
    parts = sum(np.asarray(r["part"], dtype=np.float64) for r in res.results)
    return (np.abs(parts).sum(axis=1) / (B * C)).astype(np.float32)


# revision 8
# speedup vs baseline: 1.3252x; 1.1488x over previous
"""MDCA loss kernel for Trainium2, 8 NeuronCores, data-parallel over batch.

reference:
    counts[c]   = histogram(target) ; avg_count = counts/B
    avg_conf    = mean(logits, axis=1)            # [E, C]
    loss[e]     = mean_c |avg_conf[e,c] - avg_count[c]|

Strategy per core (batch shard of 1024 rows, partition p holds rows 8p..8p+7):
  - the 16.4 MB logits shard streams over the TWO HWDGE queues only
    (sync + scalar).  SWDGE (gpsimd) is avoided for bulk data: its
    SBUF-resident descriptor rings sit on AXI ports shared with SDMA
    engines 7/15, which then lag and straggle the stream end by ~4 us.
    16 KB lines (4 contiguous rows/partition) hit the engines' best
    per-descriptor rate; the HBM QoS governor caps the aggregate at
    ~330 GB/s after ~15 us — that is the wire floor.  All dma_starts are
    emitted first so both rings stay full end-to-end.
  - SDMA round-robin is packet-fair, so ring position == arrival order:
    scalar's ring (40 packet-rounds) empties before sync's (56 rounds),
    and sync ends with four 0.5 MB single-row closers DMA'd straight
    into f32r tiles (bitcast).  Closers need NO DVE fold — just two
    ~390 ns PE matmuls each — so every fold of the 2 MB chunks happens
    while the closers drain, and the post-stream tail is ~1 us of PE.
  - target arrives as [128, 128] f32 (each row-target replicated 16x ->
    512 B lines, no read-modify-write descriptors) at the head of
    scalar's ring; iota runs on the otherwise-idle gpsimd immediately,
    so the histogram phase finishes ~20 us before the first conf matmul
  - ONE PSUM accumulation chain per 500-column half: opened by the
    histogram matmuls ([128,4] -1 weights broadcast -count to the 4 exit
    rows), continued by f32r conf matmuls ([128,4] selector with ones in
    column e folds rows+partitions), closed by the last closer:
    psum[e,c] = sum_conf[e,c] - count[c] with no combine step
  - DVE folds 4-row chunks pairwise into [128, 1000] f32r tiles
  - tail: 2 matmuls + 2 tiny PSUM->SBUF copies + two parallel 8 KB
    stores, one per HWDGE ring
  - host sums the 8 per-core partials and takes |.|-mean / (B*C) -> loss[4]
    (an on-device AllReduce costs ~35 us for 16 KB; host finish wins)
"""

import os
import sys

for _p in ("/opt/trn_rl_repo", "/root/.axon_site/_ro/trn_rl_repo"):
    if os.path.isdir(_p) and _p not in sys.path:
        sys.path.insert(0, _p)

import numpy as np

import concourse.bass as bass
import concourse.bacc as bacc
import concourse.tile as tile
import concourse.mybir as mybir
from concourse.bass_utils import run_bass_kernel_spmd

E, B, C = 4, 8192, 1000
N_CORES = 8
BS = B // N_CORES          # 1024 batch rows per core
GP = 8                     # rows folded per partition (BS = 128 * GP)
CH = C // 2                # 500, C half per PSUM bank
REP = 16                   # target replication -> 512 B DMA lines
F32 = mybir.dt.float32
F32R = mybir.dt.float32r
BF16 = mybir.dt.bfloat16


def build_nc():
    nc = bacc.Bacc(
        "TRN2",
        target_bir_lowering=False,
        debug=False,
        enable_asserts=False,
        num_devices=N_CORES,
    )

    logits = nc.dram_tensor("logits", [E, BS, C], F32, kind="ExternalInput")
    target = nc.dram_tensor("target_f", [128, GP * REP], F32,
                            kind="ExternalInput")
    part_out = nc.dram_tensor("part", [E, C], F32, kind="ExternalOutput")

    # per-exit view: partition p holds rows 8p..8p+7
    src = [logits[e].rearrange("(p i) c -> p i c", i=GP) for e in range(E)]

    with tile.TileContext(nc) as tc:
        with (
            tc.tile_pool(name="const", bufs=1) as const,
            tc.tile_pool(name="ld2", bufs=7) as ld2,
            tc.tile_pool(name="ldz", bufs=4) as ldz,
            tc.tile_pool(name="fold", bufs=4) as foldp,
            tc.tile_pool(name="work", bufs=3) as work,
            tc.tile_pool(name="psum", bufs=1, space=bass.MemorySpace.PSUM) as psum,
        ):
            # ---- phase 1: every load DMA first, so both HWDGE rings
            # fill immediately and drain back-to-back
            def ld_dma(eng, pool, tag, e, r0, r1, dt=F32):
                rows = r1 - r0
                t = pool.tile([128, rows * C], dt, tag=tag,
                              name=f"{tag}_e{e}r{r0}")
                in_ = src[e][:, r0:r1, :]
                if dt is F32R:
                    in_ = in_.bitcast(F32R)
                eng.dma_start(
                    out=t.rearrange("p (i c) -> p i c", i=rows), in_=in_
                )
                return t

            # sync ring: 3x 2MB + 4 single-row closers (56 packet-rounds)
            tA_s = ld_dma(nc.sync, ld2, "ld2", 0, 0, 4)
            tB_s = ld_dma(nc.sync, ld2, "ld2", 1, 4, 8)
            tD_s = ld_dma(nc.sync, ld2, "ld2", 3, 0, 4)
            tZ = [ld_dma(nc.sync, ldz, "ldz", 3, 4 + i, 5 + i, F32R)
                  for i in range(4)]

            # scalar ring: target + 4x 2MB (40 packet-rounds)
            tgt_sb = const.tile([128, GP * REP], F32, tag="tgt")
            nc.scalar.dma_start(out=tgt_sb[:], in_=target[:])
            tA_c = ld_dma(nc.scalar, ld2, "ld2", 1, 0, 4)
            tB_c = ld_dma(nc.scalar, ld2, "ld2", 2, 4, 8)
            tC_c = ld_dma(nc.scalar, ld2, "ld2", 0, 4, 8)
            tE_c = ld_dma(nc.scalar, ld2, "ld2", 2, 0, 4)

            # ---- phase 2: constants (DVE + idle gpsimd)
            sels_f = const.tile([128, 4 * E], F32, tag="sels_f")
            nc.vector.memset(sels_f[:], 0.0)
            for e in range(E):
                nc.vector.memset(sels_f[:, 4 * e + e : 4 * e + e + 1], 1.0)
            sels = const.tile([128, 4 * E], F32R, tag="sels")
            nc.vector.tensor_copy(sels[:], sels_f[:])
            neg1 = const.tile([128, E], BF16, tag="neg1")
            nc.vector.memset(neg1[:], -1.0)
            iota_f = const.tile([128, C], F32, tag="iota")
            nc.gpsimd.iota(
                iota_f[:],
                pattern=[[1, C]],
                base=0,
                channel_multiplier=0,
                allow_small_or_imprecise_dtypes=True,
            )

            # one merged PSUM accumulation chain per column half
            pbank = [
                psum.tile([E, CH], F32, tag=f"pc{h}", name=f"pc{h}")
                for h in range(2)
            ]

            # ---- phase 3: histogram one-hots + chain-opening matmuls
            for i in range(GP):
                onehot = work.tile([128, C], BF16, tag="onehot")
                nc.vector.tensor_scalar(
                    onehot[:],
                    iota_f[:],
                    tgt_sb[:, REP * i : REP * i + 1],
                    None,
                    mybir.AluOpType.is_equal,
                )
                for h in range(2):
                    nc.tensor.matmul(
                        pbank[h][:],
                        neg1[:],
                        onehot[:, h * CH : (h + 1) * CH],
                        start=(i == 0),
                        stop=False,
                    )

            # ---- phase 4: folds + conf matmuls in expected arrival order
            def mm(f, e, stop=False):
                for h in range(2):
                    nc.tensor.matmul(
                        pbank[h][:],
                        sels[:, 4 * e : 4 * e + 4],
                        f[:, h * CH : (h + 1) * CH],
                        start=False,
                        stop=stop,
                    )

            def fold4(t, e):
                for g in range(2):
                    f = foldp.tile([128, C], F32R, tag="fold",
                                   name=f"f_e{e}g{g}")
                    nc.vector.tensor_add(
                        f[:], t[:, 2 * g * C : (2 * g + 1) * C],
                        t[:, (2 * g + 1) * C : (2 * g + 2) * C],
                    )
                    mm(f, e)

            # ring-position arrival order:
            #   A_s@8, A_c@16, B_s@16, B_c@24, D_s@24, C_c@32, Z1@32,
            #   Z2@40, E_c@40, Z3@48, Z4@56  (packet-rounds)
            fold4(tA_s, 0)
            fold4(tA_c, 1)
            fold4(tB_s, 1)
            fold4(tB_c, 2)
            fold4(tD_s, 3)
            fold4(tC_c, 0)
            mm(tZ[0], 3)
            mm(tZ[1], 3)
            fold4(tE_c, 2)
            mm(tZ[2], 3)
            mm(tZ[3], 3, stop=True)

            # ---- phase 5: PSUM->SBUF copies + parallel 8 KB stores
            sb0 = work.tile([E, CH], F32, tag="sb0")
            nc.vector.tensor_copy(sb0[:], pbank[0][:])
            nc.sync.dma_start(out=part_out[:, 0:CH], in_=sb0[:])
            sb1 = work.tile([E, CH], F32, tag="sb1")
            nc.vector.tensor_copy(sb1[:], pbank[1][:])
            nc.scalar.dma_start(out=part_out[:, CH:C], in_=sb1[:])

    nc.compile()
    return nc


_NC_CACHE = {}


def _get_nc():
    if "nc" not in _NC_CACHE:
        _NC_CACHE["nc"] = build_nc()
    return _NC_CACHE["nc"]


def make_in_maps(logits: np.ndarray, target: np.ndarray):
    logits = np.ascontiguousarray(logits, dtype=np.float32)
    target = np.asarray(target)
    in_maps = []
    for c in range(N_CORES):
        lg = logits[:, c * BS : (c + 1) * BS, :]
        tg = np.repeat(
            target[c * BS : (c + 1) * BS].astype(np.float32).reshape(128, GP),
            REP, axis=1,
        )
        in_maps.append(
            {"logits": np.ascontiguousarray(lg),
             "target_f": np.ascontiguousarray(tg)}
        )
    return in_maps


def kernel(logits: np.ndarray, target: np.ndarray) -> np.ndarray:
    nc = _get_nc()
    in_maps = make_in_maps(logits, target)
    res = run_bass_kernel_spmd(nc, in_maps, core_ids=list(range(N_CORES)))
    parts = sum(np.asarray(r["part"], dtype=np.float64) for r in res.results)
    return (np.abs(parts).sum(axis=1) / (B * C)).astype(np.float32)
